# revision 8
# baseline (speedup 1.0000x reference)
"""GQA attention (dense_transformer) on 8 TRN2 NeuronCores.

Sharding: core c = b*4 + j  (b = batch 0..1, j = tensor-parallel rank 0..3).
Each core computes q-heads 8j..8j+7 (kv heads 2j, 2j+1) for batch b, then an
AllGather of attn^T over the 4 ranks of its batch group, then its 512-column
shard of the output projection.  Host assembles the full output.

All big matmuls run in float32r (full-rate PE, ~1e-4 rel precision).
The AllGather + wo tail runs in bf16.

Self-contained: hardcodes shapes from the problem spec.
"""
import os
import sys

sys.path.insert(0, "/opt/trn_rl_repo")

import numpy as np
import ml_dtypes

import concourse.bass as bass
import concourse.mybir as mybir
import concourse.tile as tile
from concourse import bacc
from concourse.bass_utils import run_bass_kernel_spmd
from concourse.masks import make_identity

HIDDEN = 2048
N_HEADS = 32
N_KV_HEADS = 8
HEAD_DIM = 64
B_FULL, T_FULL = 2, 2048

NCORES = 8
NTP = 4                       # tensor-parallel ranks per batch group
NHL = N_HEADS // NTP          # 8 local q heads
NKVL = N_KV_HEADS // NTP      # 2 local kv heads
QF = NHL * HEAD_DIM           # 512 local q features
KF = NKVL * HEAD_DIM          # 128 local kv features
COLS = HIDDEN // NTP          # 512 output columns per rank
TC = 256                      # t-chunk width
P = 128
TPC = TC // P                 # tk tiles per chunk

F32 = mybir.dt.float32
F32R = mybir.dt.float32r
BF16 = mybir.dt.bfloat16

SCALE = 1.0 / np.sqrt(HEAD_DIM)

LAST_EXEC_NS = None
LAST_RESULTS = None


def build_kernel(T=T_FULL):
    """One SPMD program; every core runs the same code on its shard."""
    assert T % TC == 0
    NCH = T // TC             # t-chunks
    KH = HIDDEN // P          # 16 k-tiles over hidden
    NTT = T // P              # tk tiles total

    nc = bacc.Bacc("TRN2", debug=False)

    xT = nc.dram_tensor("xT", [HIDDEN, T], F32R, kind="ExternalInput")
    wqT = nc.dram_tensor("wqT", [HIDDEN, QF], F32R, kind="ExternalInput")
    wkT = nc.dram_tensor("wkT", [HIDDEN, KF], F32R, kind="ExternalInput")
    wvT = nc.dram_tensor("wvT", [HIDDEN, KF], F32R, kind="ExternalInput")
    woT = nc.dram_tensor("woT", [HIDDEN, COLS], BF16, kind="ExternalInput")
    cosT = nc.dram_tensor("cosT", [P, T], F32R, kind="ExternalInput")
    sinTs = nc.dram_tensor("sinTs", [P, T], F32R, kind="ExternalInput")
    swp = nc.dram_tensor("swp", [P, P], F32R, kind="ExternalInput")
    msk = nc.dram_tensor("msk", [P, TPC * TC], F32R, kind="ExternalInput")
    out = nc.dram_tensor("out", [COLS, T], F32, kind="ExternalOutput")

    cc_in = [nc.dram_tensor(f"cc_in{c}", [QF, TC], BF16) for c in range(NCH)]
    cc_out = [
        nc.dram_tensor(f"cc_out{c}", [NTP * QF, TC], BF16) for c in range(NCH)
    ]
    groups = [[0, 1, 2, 3], [4, 5, 6, 7]]

    from contextlib import ExitStack
    with tile.TileContext(nc) as tc, ExitStack() as est:
        consts = est.enter_context(tc.tile_pool(name="consts", bufs=1))
        kpool = est.enter_context(tc.tile_pool(name="kpool", bufs=1))
        xcpool = est.enter_context(tc.tile_pool(name="xcpool", bufs=18))
        stream = est.enter_context(tc.tile_pool(name="stream", bufs=3))
        qrpool = est.enter_context(tc.tile_pool(name="qrpool", bufs=6))
        ppool = est.enter_context(tc.tile_pool(name="ppool", bufs=6))
        atpool = est.enter_context(tc.tile_pool(name="atpool", bufs=6))
        agpool = est.enter_context(tc.tile_pool(name="agpool", bufs=18))
        small = est.enter_context(tc.tile_pool(name="small", bufs=3))
        ps_proj = est.enter_context(tc.tile_pool(name="ps_proj", bufs=2, space="PSUM"))
        ps_s = est.enter_context(tc.tile_pool(name="ps_s", bufs=2, space="PSUM"))
        ps_pv = est.enter_context(tc.tile_pool(name="ps_pv", bufs=2, space="PSUM"))
        ps_misc = est.enter_context(tc.tile_pool(name="ps_misc", bufs=2, space="PSUM"))

        # ---- constants ----
        cos_sb = consts.tile([P, T], F32R)
        sin_sb = consts.tile([P, T], F32R)
        swp_sb = consts.tile([P, P], F32R)
        msk_sb = consts.tile([P, TPC, TC], F32R)
        id_sb = consts.tile([P, P], F32R)
        id_f32 = consts.tile([P, P], F32)
        ones_sb = consts.tile([1, HEAD_DIM], F32R)
        ones_f32 = consts.tile([P, 1], F32)
        ones_row_f32 = consts.tile([1, HEAD_DIM], F32)
        wq_sb = consts.tile([P, KH, QF], F32R)
        wk_sb = consts.tile([P, KH, KF], F32R)
        wv_sb = consts.tile([P, KH, KF], F32R)
        wo_sb = consts.tile([P, KH, COLS], BF16)

        for c in range(T // 512):
            sl = slice(c * 512, (c + 1) * 512)
            nc.sync.dma_start(out=cos_sb[:, sl], in_=cosT[:, sl])
            nc.sync.dma_start(out=sin_sb[:, sl], in_=sinTs[:, sl])
        nc.sync.dma_start(out=swp_sb, in_=swp[:, :])
        nc.sync.dma_start(out=msk_sb,
                          in_=msk[:, :].rearrange("p (o q) -> p o q", o=TPC))
        make_identity(nc, id_f32)
        nc.vector.tensor_copy(id_sb, id_f32)
        nc.vector.memset(ones_f32, 1.0)
        nc.vector.memset(ones_row_f32, 1.0)
        nc.vector.tensor_copy(ones_sb, ones_row_f32)
        wqv = wqT[:, :].rearrange("(t p) f -> p t f", p=P)
        wkv = wkT[:, :].rearrange("(t p) f -> p t f", p=P)
        wvv = wvT[:, :].rearrange("(t p) f -> p t f", p=P)
        wov = woT[:, :].rearrange("(t p) f -> p t f", p=P)
        for k in range(KH):
            nc.sync.dma_start(out=wq_sb[:, k, :], in_=wqv[:, k, :])
            nc.sync.dma_start(out=wk_sb[:, k, :], in_=wkv[:, k, :])
            nc.sync.dma_start(out=wv_sb[:, k, :], in_=wvv[:, k, :])
            nc.sync.dma_start(out=wo_sb[:, k, :], in_=wov[:, k, :])

        # ---- persistent K / V accumulators ----
        # KA = [g0; g0] roped K^T, KB = [g1; g1]  (dup so head-parity base
        # partitions always match between lhsT and rhs in the scores matmul)
        KA = kpool.tile([P, T], F32R, tag="KA")
        KB = kpool.tile([P, T], F32R, tag="KB")
        # V natural layout per tk-tile: cols = [V_g0 (64) | 1 | V_g1 (64) | 1]
        vaug = kpool.tile([P, NTT, 2 * HEAD_DIM + 2], F32R, tag="vaug")
        for t in range(NTT):
            nc.vector.tensor_copy(vaug[:, t, HEAD_DIM:HEAD_DIM + 1], ones_f32)
            nc.vector.tensor_copy(vaug[:, t, 2 * HEAD_DIM + 1:2 * HEAD_DIM + 2],
                                  ones_f32)

        xv = xT[:, :].rearrange("(t p) n -> p t n", p=P)

        def rope(raw_sb, cs, ss, out_tile):
            """out = raw*cos + swap(raw)*sin_signed  (all [P, TC])."""
            sw_ps = ps_misc.tile([P, TC], F32, tag="misc")
            nc.tensor.matmul(sw_ps, lhsT=swp_sb, rhs=raw_sb, start=True, stop=True)
            m2 = stream.tile([P, TC], F32R, tag="tmp")
            nc.vector.tensor_tensor(out=m2, in0=sw_ps, in1=ss, op=mybir.AluOpType.mult)
            nc.vector.tensor_tensor(out=out_tile, in0=raw_sb, in1=cs,
                                    op=mybir.AluOpType.mult)
            nc.vector.tensor_tensor(out=out_tile, in0=out_tile, in1=m2,
                                    op=mybir.AluOpType.add)

        for c in range(NCH):
            csl = slice(c * TC, (c + 1) * TC)
            cs = cos_sb[:, csl]
            ss = sin_sb[:, csl]

            # ---- load x chunk ----
            xc = []
            for k in range(KH):
                t_ = xcpool.tile([P, TC], F32R, tag="xc")
                nc.sync.dma_start(out=t_, in_=xv[:, k, csl])
                xc.append(t_)

            # ---- Q projection + rope: 4 head-pair tiles [128, TC] ----
            qrope = []
            for m in range(4):
                q_ps = ps_proj.tile([P, TC], F32, tag="proj")
                for k in range(KH):
                    nc.tensor.matmul(q_ps, lhsT=wq_sb[:, k, m * P:(m + 1) * P],
                                     rhs=xc[k], start=(k == 0), stop=(k == KH - 1))
                raw = stream.tile([P, TC], F32R, tag="raw")
                nc.scalar.activation(out=raw, in_=q_ps,
                                     func=mybir.ActivationFunctionType.Copy)
                qt = qrpool.tile([P, TC], F32R, tag="qrope")
                rope(raw, cs, ss, qt)
                qrope.append(qt)

            # ---- K projection + rope + dup ----
            k_ps = ps_proj.tile([P, TC], F32, tag="proj")
            for k in range(KH):
                nc.tensor.matmul(k_ps, lhsT=wk_sb[:, k, :], rhs=xc[k],
                                 start=(k == 0), stop=(k == KH - 1))
            kraw = stream.tile([P, TC], F32R, tag="raw")
            nc.scalar.activation(out=kraw, in_=k_ps,
                                 func=mybir.ActivationFunctionType.Copy)
            krope = stream.tile([P, TC], F32R, tag="raw")
            rope(kraw, cs, ss, krope)
            nc.vector.tensor_copy(KA[0:64, csl], krope[0:64, :])
            nc.vector.tensor_copy(KA[64:128, csl], krope[0:64, :])
            nc.vector.tensor_copy(KB[0:64, csl], krope[64:128, :])
            nc.vector.tensor_copy(KB[64:128, csl], krope[64:128, :])

            # ---- V projection (as V^T) + transpose into vaug ----
            v_ps = ps_proj.tile([P, TC], F32, tag="proj")
            for k in range(KH):
                nc.tensor.matmul(v_ps, lhsT=wv_sb[:, k, :], rhs=xc[k],
                                 start=(k == 0), stop=(k == KH - 1))
            vt = stream.tile([P, TC], F32R, tag="raw")
            nc.scalar.activation(out=vt, in_=v_ps,
                                 func=mybir.ActivationFunctionType.Copy)
            for tt in range(TPC):
                tp_ps = ps_misc.tile([P, P], F32R, tag="misc")
                nc.tensor.transpose(tp_ps, vt[:, tt * P:(tt + 1) * P], id_sb)
                tkt = c * TPC + tt
                nc.vector.tensor_copy(vaug[:, tkt, 0:HEAD_DIM], tp_ps[:, 0:HEAD_DIM])
                nc.vector.tensor_copy(vaug[:, tkt, HEAD_DIM + 1:2 * HEAD_DIM + 1],
                                      tp_ps[:, HEAD_DIM:2 * HEAD_DIM])

            # ---- attention for this t-chunk ----
            n_tk = (c + 1) * TPC
            at_tiles = []
            for _ in range(4):
                at_t = atpool.tile([P, TC], BF16, tag="attnT")
                at_tiles.append(at_t)
            for h in range(NHL):
                g = h // (NHL // NKVL)        # local kv group (h // 4)
                par = h % 2
                base = par * HEAD_DIM
                ksrc = KA if g == 0 else KB
                qt = qrope[h // 2]
                lsl = slice(base, base + HEAD_DIM)

                pv_ps = ps_pv.tile([HEAD_DIM + 1, TC], F32, tag="pv")
                for i in range(n_tk):
                    s_ps = ps_s.tile([P, TC], F32, tag="s")
                    nc.tensor.matmul(
                        s_ps,
                        lhsT=ksrc[lsl, i * P:(i + 1) * P],
                        rhs=qt[lsl, :],
                        start=True, stop=True)
                    p_sb = ppool.tile([P, TC], F32R, tag="p")
                    nc.scalar.activation(out=p_sb, in_=s_ps,
                                         func=mybir.ActivationFunctionType.Exp,
                                         scale=float(SCALE))
                    o = i - c * TPC
                    if o >= 0:
                        nc.vector.tensor_tensor(out=p_sb, in0=p_sb,
                                                in1=msk_sb[:, o, :],
                                                op=mybir.AluOpType.mult)
                    vsl = slice(g * (HEAD_DIM + 1), (g + 1) * (HEAD_DIM + 1))
                    nc.tensor.matmul(pv_ps, lhsT=vaug[:, i, vsl], rhs=p_sb,
                                     start=(i == 0), stop=(i == n_tk - 1))

                # normalize: attnT rows = pv[0:64] * (1/sum) broadcast
                rec = small.tile([1, TC], F32R, tag="recip")
                with nc.allow_low_precision(reason="f32r softmax denom"):
                    nc.vector.reciprocal(rec, pv_ps[HEAD_DIM:HEAD_DIM + 1, :])
                rep_ps = ps_misc.tile([HEAD_DIM, TC], F32, tag="misc")
                nc.tensor.matmul(rep_ps, lhsT=ones_sb, rhs=rec, start=True, stop=True)
                rep = small.tile([HEAD_DIM, TC], F32, tag="rep")
                nc.scalar.activation(out=rep, in_=rep_ps,
                                     func=mybir.ActivationFunctionType.Copy)
                nc.vector.tensor_tensor(
                    out=at_tiles[h // 2][base:base + HEAD_DIM, :],
                    in0=pv_ps[0:HEAD_DIM, :], in1=rep, op=mybir.AluOpType.mult)

            # ---- AllGather attn^T chunk across the 4 TP ranks ----
            for m in range(4):
                nc.sync.dma_start(out=cc_in[c][m * P:(m + 1) * P, :],
                                  in_=at_tiles[m])
            nc.gpsimd.collective_compute(
                "AllGather", mybir.AluOpType.bypass,
                replica_groups=groups,
                ins=[cc_in[c][:, :]],
                outs=[cc_out[c][:, :]],
            )
            ccv = cc_out[c][:, :].rearrange("(t p) n -> p t n", p=P)
            ag = []
            for k in range(KH):
                t_ = agpool.tile([P, TC], BF16, tag="ag")
                nc.sync.dma_start(out=t_, in_=ccv[:, k, :])
                ag.append(t_)

            # ---- output projection shard: y^T[cols, t-chunk] ----
            for m in range(4):
                y_ps = ps_proj.tile([P, TC], F32, tag="proj")
                for k in range(KH):
                    nc.tensor.matmul(y_ps, lhsT=wo_sb[:, k, m * P:(m + 1) * P],
                                     rhs=ag[k], start=(k == 0), stop=(k == KH - 1))
                y_sb = small.tile([P, TC], F32, tag="ysb")
                nc.scalar.activation(out=y_sb, in_=y_ps,
                                     func=mybir.ActivationFunctionType.Copy)
                nc.sync.dma_start(out=out[m * P:(m + 1) * P, csl], in_=y_sb)

    nc.compile()
    return nc


_NC_CACHE = {}


def _get_nc(T):
    if T not in _NC_CACHE:
        _NC_CACHE[T] = build_kernel(T)
    return _NC_CACHE[T]


def _perm64():
    """Per-head permutation: interleaved (even,odd) -> [r(32) | i(32)]."""
    p = np.empty(HEAD_DIM, dtype=np.int64)
    p[:32] = np.arange(0, HEAD_DIM, 2)
    p[32:] = np.arange(1, HEAD_DIM, 2)
    return p


def make_inputs(x, freqs_cis, wq, wk, wv, wo, T):
    """Build the 8 per-core input maps (host-side sharding + layout prep)."""
    perm = _perm64()
    f32 = np.float32

    cos = np.asarray(freqs_cis[:T, :, 0], dtype=f32)   # [T, 32]
    sin = np.asarray(freqs_cis[:T, :, 1], dtype=f32)
    cosT = np.tile(cos.T, (4, 1)).astype(f32)                        # [128, T]
    sinTs = np.tile(np.vstack([-sin.T, sin.T]), (2, 1)).astype(f32)  # [128, T]

    J = np.zeros((HEAD_DIM, HEAD_DIM), dtype=f32)
    J[np.arange(32), np.arange(32) + 32] = 1.0
    J[np.arange(32) + 32, np.arange(32)] = 1.0
    swp = np.zeros((P, P), dtype=f32)
    swp[:HEAD_DIM, :HEAD_DIM] = J
    swp[HEAD_DIM:, HEAD_DIM:] = J

    # causal diag masks [128, TPC*TC]: msk[p, o*TC + q] = (q >= p + 128*o)
    q_idx = np.arange(TC)
    p_idx = np.arange(P)[:, None]
    msk = np.concatenate(
        [(q_idx[None, :] >= p_idx + P * o).astype(f32) for o in range(TPC)],
        axis=1)

    def permute_heads(w, n_heads):
        wh = np.asarray(w, f32).reshape(n_heads, HEAD_DIM, HIDDEN)
        return wh[:, perm, :].reshape(n_heads * HEAD_DIM, HIDDEN)

    wq_p = permute_heads(wq, N_HEADS)
    wk_p = permute_heads(wk, N_KV_HEADS)
    wv_n = np.asarray(wv, f32)
    wo_n = np.asarray(wo, f32)

    in_maps = []
    for core in range(NCORES):
        b, j = divmod(core, NTP)
        xTc = np.ascontiguousarray(np.asarray(x[b, :T], f32).T)     # [H, T]
        wqTc = np.ascontiguousarray(wq_p[j * QF:(j + 1) * QF].T)    # [H, QF]
        wkTc = np.ascontiguousarray(wk_p[j * KF:(j + 1) * KF].T)
        wvTc = np.ascontiguousarray(wv_n[j * KF:(j + 1) * KF].T)
        woTc = np.ascontiguousarray(wo_n[j * COLS:(j + 1) * COLS].T).astype(
            ml_dtypes.bfloat16)                                     # [H, COLS]
        in_maps.append({
            "xT": xTc, "wqT": wqTc, "wkT": wkTc, "wvT": wvTc, "woT": woTc,
            "cosT": cosT, "sinTs": sinTs, "swp": swp, "msk": msk,
        })
    return in_maps


def kernel(x, freqs_cis, wq, wk, wv, wo):
    global LAST_EXEC_NS, LAST_RESULTS
    T = x.shape[1]
    nc = _get_nc(T)
    in_maps = make_inputs(x, freqs_cis, wq, wk, wv, wo, T)
    trace = bool(int(os.environ.get("KERNEL_TRACE", "0")))
    res = run_bass_kernel_spmd(nc, in_maps, core_ids=list(range(NCORES)),
                               trace=trace)
    LAST_EXEC_NS = res.exec_time_ns
    LAST_RESULTS = res
    out = np.empty((B_FULL, T, HIDDEN), dtype=np.float32)
    for core in range(NCORES):
        b, j = divmod(core, NTP)
        out[b, :, j * COLS:(j + 1) * COLS] = res.results[core]["out"].T
    return out
